# revision 5
# baseline (speedup 1.0000x reference)
"""Trainium2 Bass kernel for nn_AttentionUnit (B=4, S=2048, D=1024, H=16).

Sharding: 8 cores = 4 batches x 2 head-groups (Megatron column/row split).
Each core receives its batch's q/k/v (bf16, pre-transposed, seq-chunked)
plus its weight halves, and returns an unnormalized-pair partial
[2048, 1024] bf16; the host sums the two head-group partials per batch.

Per core (batch b, 8-head half hg):
  Q^T,K^T = (Wq/Wk half)^T-proj of inputs   [dh=512 on partitions, seq free]
  V       = natural [seq, dh=512] (+ pad-indicator column per head used as
            the softmax-denominator ones column); padded key rows of V are
            zeroed at projection time (V-side key-padding: padded keys then
            contribute 0 to both numerator and denominator, so no exp bias
            is needed).
  S^T     = K @ Q^T / 8 (causal blocks skipped; causal diag via mask add)
  P^T     = exp(S^T)  (unnormalized, bf16, one ACT instruction per PAIR of
            k-tiles: score pairs land in a 2-bank PSUM tile so each exp
            covers 1024 columns, halving ACT fixed cost and sem traffic)
  O^T     = V_aug^T @ P^T  -> row 64 is the softmax denominator
  attn^T  = O^T[0:64] * recip(denom)  (reciprocal_approx_fast, broadcast
            across partitions via one-hot [8,128] select-matmuls straight
            from the recip rows)
  partial = attn @ Wo_half -> bf16 -> host

All matmuls bf16 with fp32 PSUM accumulation; softmax entirely fp32.
Engine split: PE matmuls; ACT exp; DVE bias-adds/AT copies/normalize/recip;
GpSimd causal-mask adds, V pad-multiplies and output-tile casts.
"""

import sys

sys.path.insert(0, "/opt/trn_rl_repo")

import numpy as np
import ml_dtypes

S = 2048
D = 1024
P = 128
DH = 64          # head dim
HPC = 8          # heads per core
DHH = 512        # dh per core (8 heads * 64)
QB = 512         # q block
NQB = S // QB    # 4
DMC = D // P     # 8 dmodel chunks
NT = S // P      # 16 k tiles
NEG_CAUSAL = -1.0e12   # added pre-scale (scale=0.125 applied inside exp)

_CACHE = {}


def _calibrate_scheduler_model():
    """Calibrate the tile list-scheduler's hardware cost model to measured
    TRN2 rates (defaults model the PE at its mid p-state and ACT/DVE too
    fast, which makes the compile-time schedule under-hoist independent
    work across cross-engine dependency hops). Measured on this part:
    back-to-back 512-col bf16 matmuls stream at 216.7ns (2.4GHz), exp of a
    [128,1024] PSUM pair takes ~1114ns, [128,512] DVE fp32 ops ~750ns, and
    cross-engine semaphore hops ~300ns+."""
    from concourse import hw_specs, mybir

    sp = hw_specs.TRN2Spec
    sp.PE_CYCLE_PSTATE_MID = sp.PE_CYCLE
    sp.PE_CYCLE_PSTATE_LOW = sp.PE_CYCLE
    sp.CYCLE_T[mybir.EngineType.Activation] = 1e9 / 1.0e9
    sp.CYCLE_T[mybir.EngineType.DVE] = 1e9 / 0.85e9
    sp.SEM_DELAY = 300


def _build_program(kcap=NT):
    import concourse.bass as bass
    import concourse.tile as tile
    from concourse import bacc, mybir

    _calibrate_scheduler_model()

    f32 = mybir.dt.float32
    bf16 = mybir.dt.bfloat16
    ADD = mybir.AluOpType.add
    MUL = mybir.AluOpType.mult
    EXP = mybir.ActivationFunctionType.Exp

    nc = bacc.Bacc("TRN2", target_bir_lowering=False, debug=False)

    # --- external I/O ---
    x0_d = nc.dram_tensor("x0", [NQB, 3, D, QB], bf16, kind="ExternalInput")
    wq_d = nc.dram_tensor("wq", [D, DHH], bf16, kind="ExternalInput")
    wk_d = nc.dram_tensor("wk", [D, DHH], bf16, kind="ExternalInput")
    wv_d = nc.dram_tensor("wv", [D, DHH], bf16, kind="ExternalInput")
    wo_d = nc.dram_tensor("wo", [DHH, D], bf16, kind="ExternalInput")
    vpad_d = nc.dram_tensor("vpad", [P, NT], f32, kind="ExternalInput")
    vpadc_d = nc.dram_tensor("vpadc", [P, NT, HPC, 1], bf16, kind="ExternalInput")
    mask_d = nc.dram_tensor("mask", [P, P], f32, kind="ExternalInput")
    bq_d = nc.dram_tensor("bq", [P, 4], f32, kind="ExternalInput")
    sel8_d = nc.dram_tensor("sel8", [HPC, 4, P], bf16, kind="ExternalInput")
    bk_d = nc.dram_tensor("bk", [P, 4], f32, kind="ExternalInput")
    out_d = nc.dram_tensor("outp", [S, D], bf16, kind="ExternalOutput")

    with tile.TileContext(nc) as tc:
        with (
            tc.tile_pool(name="const", bufs=1) as constp,
            tc.tile_pool(name="inp", bufs=2) as inp,
            tc.tile_pool(name="probs", bufs=3) as probsp,
            tc.tile_pool(name="small", bufs=2) as smallp,
            tc.tile_pool(name="osb", bufs=2) as osbp,
            tc.tile_pool(name="psAC", bufs=2, space="PSUM") as psAC,
            tc.tile_pool(name="psB", bufs=2, space="PSUM") as psB,
            tc.tile_pool(name="psS", bufs=2, space="PSUM") as psS,
        ):
            # ---- persistent SBUF tensors ----
            wq_sb = constp.tile([P, DMC, DHH], bf16, tag="wq")
            wk_sb = constp.tile([P, DMC, DHH], bf16, tag="wk")
            wv_sb = constp.tile([P, DMC, DHH], bf16, tag="wv")
            wo_sb = constp.tile([P, 4, D], bf16, tag="wo")
            QT = constp.tile([P, 4, S], bf16, tag="QT")
            KT = constp.tile([P, 4, S], bf16, tag="KT")
            V = constp.tile([P, NT, HPC, DH + 1], bf16, tag="V")
            AT = constp.tile([P, 4, S], bf16, tag="AT")
            vpad = constp.tile([P, NT], f32, tag="vpad")
            mask = constp.tile([P, P], f32, tag="mask")
            bq_sb = constp.tile([P, 4], f32, tag="bq")
            bk_sb = constp.tile([P, 4], f32, tag="bk")
            sel8 = constp.tile([HPC, 4, P], bf16, tag="sel8")

            nc.sync.dma_start(wq_sb[:], wq_d.rearrange("(c p) m -> p c m", p=P))
            # pad-indicator column: the softmax-denominator "ones" column,
            # zeroed for padded key rows
            nc.sync.dma_start(V[:, :, :, DH : DH + 1], vpadc_d[:])

            def load_x(qb):
                # split q into halves so the first projection chains can
                # start before the whole 1MB q tile lands
                qinA = inp.tile([P, DMC // 2, QB], bf16, tag="qinA", name="qinA")
                qinB = inp.tile([P, DMC // 2, QB], bf16, tag="qinB", name="qinB")
                kin = inp.tile([P, DMC, QB], bf16, tag="kin", name="kin")
                vin = inp.tile([P, DMC, QB], bf16, tag="vin", name="vin")
                x0v = x0_d[qb].rearrange("t (c p) s -> t p c s", p=P)
                nc.sync.dma_start(qinA[:], x0v[0, :, 0 : DMC // 2])
                nc.sync.dma_start(qinB[:], x0v[0, :, DMC // 2 : DMC])
                if qb == 0:
                    nc.sync.dma_start(wk_sb[:], wk_d.rearrange("(c p) m -> p c m", p=P))
                nc.sync.dma_start(kin[:], x0v[1])
                if qb == 0:
                    nc.sync.dma_start(wv_sb[:], wv_d.rearrange("(c p) m -> p c m", p=P))
                nc.sync.dma_start(vin[:], x0v[2])
                if qb == 0:
                    nc.sync.dma_start(vpad[:], vpad_d[:])
                    nc.sync.dma_start(mask[:], mask_d[:])
                    nc.sync.dma_start(bq_sb[:], bq_d[:])
                    nc.sync.dma_start(bk_sb[:], bk_d[:])
                    nc.sync.dma_start(sel8[:], sel8_d[:])
                if qb == 1:
                    nc.sync.dma_start(
                        wo_sb[:], wo_d.rearrange("(c p) m -> p c m", p=P)
                    )
                return (qinA, qinB), kin, vin

            def phase_a(qb, xts):
                q0 = qb * QB
                kn = min(QB, max(0, kcap * P - q0))
                (qinA, qinB), kin, vin = xts

                for c in range(4):
                    ps = psAC.tile([P, QB], f32, tag="proj", name="ps")
                    for mc in range(DMC):
                        qsrc = qinA if mc < DMC // 2 else qinB
                        nc.tensor.matmul(
                            ps[:],
                            wq_sb[:, mc, c * P : (c + 1) * P],
                            qsrc[:, mc % (DMC // 2), :],
                            start=(mc == 0),
                            stop=(mc == DMC - 1),
                        )
                    nc.vector.tensor_scalar(
                        QT[:, c, q0 : q0 + QB], ps[:], bq_sb[:, c : c + 1], None, ADD
                    )
                    if kn > 0:
                        ps = psAC.tile([P, QB], f32, tag="proj", name="ps")
                        for mc in range(DMC):
                            nc.tensor.matmul(
                                ps[:, :kn],
                                wk_sb[:, mc, c * P : (c + 1) * P],
                                kin[:, mc, :kn],
                                start=(mc == 0),
                                stop=(mc == DMC - 1),
                            )
                        nc.vector.tensor_scalar(
                            KT[:, c, q0 : q0 + kn], ps[:, :kn], bk_sb[:, c : c + 1], None, ADD
                        )
                for j in range(4):
                    kt_i = 4 * qb + j
                    if kt_i >= kcap:
                        continue
                    ps = psAC.tile([P, QB], f32, tag="proj", name="ps")
                    for mc in range(DMC):
                        nc.tensor.matmul(
                            ps[:],
                            vin[:, mc, j * P : (j + 1) * P],
                            wv_sb[:, mc, :],
                            start=(mc == 0),
                            stop=(mc == DMC - 1),
                        )
                    # V copy fused with the key-padding zeroing
                    nc.vector.tensor_scalar(
                        V[:, kt_i, :, 0:DH],
                        ps[:].rearrange("p (h d) -> p h d", d=DH),
                        vpad[:, kt_i : kt_i + 1],
                        None,
                        MUL,
                    )

            def phase_b(qb, den):
                q0 = qb * QB
                kt_max = min(4 * qb + 4, kcap)
                # pair up k-tiles: each pair shares one 2-bank PSUM tile and
                # one exp instruction
                pairs = []
                kt = 0
                while kt < kt_max:
                    if kt + 1 < kt_max:
                        pairs.append((kt, kt + 1))
                        kt += 2
                    else:
                        pairs.append((kt, None))
                        kt += 1
                for h in range(HPC):
                    po = (h % 2) * DH
                    ch = h // 2
                    pv = psB.tile([DH + 1, QB], f32, tag="pv", name="pv")
                    for ka, kb in pairs:
                        da = max(0, P * ka - q0)
                        scp = psS.tile([P, 2, QB], f32, tag="sc", name="scp")
                        for idx, kt_i in ((0, ka), (1, kb)):
                            if kt_i is None:
                                continue
                            dd = max(0, P * kt_i - q0)
                            nc.tensor.matmul(
                                scp[:, idx, dd:],
                                KT[po : po + DH, ch, kt_i * P : (kt_i + 1) * P],
                                QT[po : po + DH, ch, q0 + dd : q0 + QB],
                                start=True,
                                stop=True,
                            )
                            if kt_i >= 4 * qb:
                                nc.vector.tensor_tensor(
                                    scp[:, idx, dd : dd + P],
                                    scp[:, idx, dd : dd + P],
                                    mask[:],
                                    ADD,
                                )
                        pr = probsp.tile([P, 2, QB], bf16, tag="probs", name="pr")
                        if kb is not None:
                            nc.scalar.activation(
                                pr[:, :, da:], scp[:, :, da:], EXP, scale=0.125
                            )
                        else:
                            nc.scalar.activation(
                                pr[:, 0, da:], scp[:, 0, da:], EXP, scale=0.125
                            )
                        for idx, kt_i in ((0, ka), (1, kb)):
                            if kt_i is None:
                                continue
                            dd = max(0, P * kt_i - q0)
                            nc.tensor.matmul(
                                pv[:, dd:],
                                V[:, kt_i, h, :],
                                pr[:, idx, dd:],
                                start=(kt_i == 0),
                                stop=(kt_i == kt_max - 1),
                            )
                    # denominator row: PSUM -> SBUF bounce (DMA cannot read
                    # PSUM), then DMA into the den row for this head
                    dtmp = smallp.tile([DH + 1, QB], f32, tag="dtmp", name="dtmp")
                    nc.vector.tensor_copy(dtmp[DH : DH + 1, :], pv[DH : DH + 1, :])
                    nc.sync.dma_start(den[h : h + 1, :], dtmp[DH : DH + 1, :])
                    if po == 0:
                        nc.vector.tensor_copy(AT[0:DH, ch, q0 : q0 + QB], pv[0:DH, :])
                    else:
                        tmp = smallp.tile([DH, QB], bf16, tag="tmp", name="tmp")
                        nc.vector.tensor_copy(tmp[:], pv[0:DH, :])
                        nc.sync.dma_start(AT[po : po + DH, ch, q0 : q0 + QB], tmp[:])

            def normalize(qb, den, recf, recd):
                q0 = qb * QB
                nc.vector.reciprocal_approx_fast(recf[:], den[:])
                with nc.allow_low_precision(reason="bf16 recip for bf16 attn"):
                    nc.vector.tensor_copy(recd[:], recf[:])
                for ch in range(4):
                    rep = psAC.tile([P, QB], f32, tag="proj", name="rep")
                    nc.tensor.matmul(
                        rep[:], sel8[:, ch, :], recd[:], start=True, stop=True
                    )
                    nc.vector.tensor_tensor(
                        AT[:, ch, q0 : q0 + QB],
                        AT[:, ch, q0 : q0 + QB],
                        rep[:],
                        MUL,
                    )

            def phase_c(qb):
                for j in range(4):
                    qt_i = 4 * qb + j
                    osb = osbp.tile([P, D], bf16, tag="osb", name="osb")
                    for half in range(2):
                        fin = psAC.tile([P, QB], f32, tag="proj", name="fin")
                        for c in range(4):
                            lhsT = AT[:, c, qt_i * P : (qt_i + 1) * P]
                            nc.tensor.matmul(
                                fin[:],
                                lhsT,
                                wo_sb[:, c, half * 512 : half * 512 + 512],
                                start=(c == 0),
                                stop=(c == 3),
                            )
                        nc.vector.tensor_copy(
                            osb[:, half * 512 : half * 512 + 512], fin[:]
                        )
                    nc.sync.dma_start(out_d[qt_i * P : (qt_i + 1) * P, :], osb[:])

            xts0 = load_x(0)
            phase_a(0, xts0)
            for qb in range(NQB):
                den = smallp.tile([HPC, QB], f32, tag=f"den{qb % 2}", name="den")
                recf = smallp.tile([HPC, QB], f32, tag=f"recf{qb % 2}", name="recf")
                recd = smallp.tile([HPC, QB], bf16, tag=f"recd{qb % 2}", name="recd")
                phase_b(qb, den)
                if qb + 1 < NQB:
                    xts = load_x(qb + 1)
                    phase_a(qb + 1, xts)
                normalize(qb, den, recf, recd)
                phase_c(qb)

    nc.compile()
    return nc


def _get_program(kcap=NT):
    if kcap not in _CACHE:
        _CACHE[kcap] = _build_program(kcap=kcap)
    return _CACHE[kcap]


def _sel8_const():
    bf = ml_dtypes.bfloat16
    sel8 = np.zeros((HPC, 4, P), dtype=bf)
    for ch in range(4):
        sel8[2 * ch, ch, 0:DH] = 1.0
        sel8[2 * ch + 1, ch, DH:P] = 1.0
    return sel8


def _make_in_maps(q_input, k_input, v_input, key_padding_mask, Wq, Wk, Wv, Wo, bq, bk):
    bf = ml_dtypes.bfloat16
    mask128 = np.where(
        np.arange(P)[None, :] < np.arange(P)[:, None], NEG_CAUSAL, 0.0
    ).astype(np.float32)
    sel8 = _sel8_const()
    xs = (q_input, k_input, v_input)
    in_maps = []
    for core in range(8):
        b = core // 2
        hg = core % 2
        sl = slice(hg * DHH, (hg + 1) * DHH)
        # key-padding as V-side zeroing: padded key rows contribute 0 to
        # both the PV numerator and the ones-column denominator
        keep = (~key_padding_mask[b]).astype(np.float32)  # [S]
        vpad = np.ascontiguousarray(keep.reshape(NT, P).T)  # [P, NT]
        vpadc = np.ascontiguousarray(
            np.broadcast_to(vpad[:, :, None, None], (P, NT, HPC, 1))
        ).astype(bf)
        x0 = np.empty((NQB, 3, D, QB), dtype=bf)
        for t in range(3):
            xT = xs[t][b].T  # [D, S] view
            for j in range(NQB):
                x0[j, t] = xT[:, j * QB : (j + 1) * QB]
        in_maps.append(
            {
                "x0": x0,
                "wq": Wq[:, sl].astype(bf),
                "wk": Wk[:, sl].astype(bf),
                "wv": Wv[:, sl].astype(bf),
                "wo": np.ascontiguousarray(Wo[sl, :]).astype(bf),
                "vpad": vpad,
                "vpadc": vpadc,
                "mask": mask128,
                "bq": np.ascontiguousarray(bq[sl].reshape(4, P).T.astype(np.float32)),
                "sel8": sel8,
                "bk": np.ascontiguousarray(bk[sl].reshape(4, P).T.astype(np.float32)),
            }
        )
    return in_maps


def run_spmd(in_maps, kcap=NT, **kwargs):
    from concourse import bass_utils

    nc = _get_program(kcap=kcap)
    return bass_utils.run_bass_kernel_spmd(
        nc, in_maps, core_ids=list(range(8)), **kwargs
    )


def kernel(q_input, k_input, v_input, key_padding_mask,
           Wq, bq, Wk, bk, Wv, bv, Wo, bo, **_unused):
    q_input = np.asarray(q_input, dtype=np.float32)
    k_input = np.asarray(k_input, dtype=np.float32)
    v_input = np.asarray(v_input, dtype=np.float32)
    key_padding_mask = np.asarray(key_padding_mask)
    in_maps = _make_in_maps(
        q_input, k_input, v_input, key_padding_mask,
        np.asarray(Wq, np.float32), np.asarray(Wk, np.float32),
        np.asarray(Wv, np.float32), np.asarray(Wo, np.float32),
        np.asarray(bq, np.float32), np.asarray(bk, np.float32),
    )
    valid = S - key_padding_mask.astype(np.int64).sum(axis=1)
    kcap = int(min(NT, max(1, -(-int(valid.max()) // P))))
    res = run_spmd(in_maps, kcap=kcap).results
    bo = np.asarray(bo, np.float32)
    bv = np.asarray(bv, np.float32)
    # bv support: normalized attention plus bv equals attn output with biased V
    # (rows of softmax sum to 1) -> fold bv through Wo into the output bias.
    extra = bv @ np.asarray(Wo, np.float32) if np.any(bv) else 0.0
    out = np.empty((4, S, D), np.float32)
    for b in range(4):
        out[b] = res[2 * b]["outp"].astype(np.float32)
        out[b] += res[2 * b + 1]["outp"].astype(np.float32)
    out += bo + extra
    return out


# revision 8
# speedup vs baseline: 1.2636x; 1.2636x over previous
"""Trainium2 Bass kernel for nn_AttentionUnit (B=4, S=2048, D=1024, H=16).

Sharding: 8 cores = 4 batches x 2 head-groups (Megatron column/row split).
Each core receives its batch's q/k/v (bf16, pre-transposed, seq-chunked)
plus its weight halves, and returns an unnormalized-pair partial
[2048, 1024] bf16; the host sums the two head-group partials per batch.

Per core (batch b, 8-head half hg):
  Q^T,K^T = (Wq/Wk half)^T-proj of inputs   [dh=512 on partitions, seq free]
  V       = natural [seq, dh=512] (+ pad-indicator column per head used as
            the softmax-denominator ones column); padded key rows of V are
            zeroed at projection time (V-side key-padding: padded keys then
            contribute 0 to both numerator and denominator, so no exp bias
            is needed).
  S^T     = K @ Q^T / 8 (causal blocks skipped; causal diag via mask add)
  P^T     = exp(S^T)  (unnormalized, bf16, one ACT instruction per PAIR of
            k-tiles: score pairs land in a 2-bank PSUM tile so each exp
            covers 1024 columns, halving ACT fixed cost and sem traffic)
  O^T     = V_aug^T @ P^T  -> row 64 is the softmax denominator
  attn^T  = O^T[0:64] * recip(denom)  (reciprocal_approx_fast, broadcast
            across partitions via one-hot [8,128] select-matmuls straight
            from the recip rows)
  partial = attn @ Wo_half -> bf16 -> host

All matmuls bf16 with fp32 PSUM accumulation; softmax entirely fp32.
Engine split: PE matmuls; ACT exp; DVE bias-adds/AT copies/normalize/recip;
GpSimd causal-mask adds, V pad-multiplies and output-tile casts.
"""

import sys

sys.path.insert(0, "/opt/trn_rl_repo")

import numpy as np
import ml_dtypes

S = 2048
D = 1024
P = 128
DH = 64          # head dim
HPC = 8          # heads per core
DHH = 512        # dh per core (8 heads * 64)
QB = 512         # q block
NQB = S // QB    # 4
DMC = D // P     # 8 dmodel chunks
NT = S // P      # 16 k tiles
NEG_CAUSAL = -1.0e12   # added pre-scale (scale=0.125 applied inside exp)

_CACHE = {}


def _build_program(kcap=NT):
    import concourse.bass as bass
    import concourse.tile as tile
    from concourse import bacc, mybir

    f32 = mybir.dt.float32
    bf16 = mybir.dt.bfloat16
    ADD = mybir.AluOpType.add
    MUL = mybir.AluOpType.mult
    EXP = mybir.ActivationFunctionType.Exp

    nc = bacc.Bacc("TRN2", target_bir_lowering=False, debug=False)

    # --- external I/O ---
    x0_d = nc.dram_tensor("x0", [NQB, 3, D, QB], bf16, kind="ExternalInput")
    wq_d = nc.dram_tensor("wq", [D, DHH], bf16, kind="ExternalInput")
    wk_d = nc.dram_tensor("wk", [D, DHH], bf16, kind="ExternalInput")
    wv_d = nc.dram_tensor("wv", [D, DHH], bf16, kind="ExternalInput")
    wo_d = nc.dram_tensor("wo", [DHH, D], bf16, kind="ExternalInput")
    vpad_d = nc.dram_tensor("vpad", [P, NT], f32, kind="ExternalInput")
    vpadc_d = nc.dram_tensor("vpadc", [P, NT, HPC, 1], bf16, kind="ExternalInput")
    mask_d = nc.dram_tensor("mask", [P, P], f32, kind="ExternalInput")
    bq_d = nc.dram_tensor("bq", [P, 4], f32, kind="ExternalInput")
    sel8_d = nc.dram_tensor("sel8", [HPC, 4, P], bf16, kind="ExternalInput")
    bk_d = nc.dram_tensor("bk", [P, 4], f32, kind="ExternalInput")
    out_d = nc.dram_tensor("outp", [S, D], bf16, kind="ExternalOutput")

    with tile.TileContext(nc) as tc:
        with (
            tc.tile_pool(name="const", bufs=1) as constp,
            tc.tile_pool(name="inp", bufs=2) as inp,
            tc.tile_pool(name="probs", bufs=5) as probsp,
            tc.tile_pool(name="small", bufs=2) as smallp,
            tc.tile_pool(name="osb", bufs=2) as osbp,
            tc.tile_pool(name="psAC", bufs=2, space="PSUM") as psAC,
            tc.tile_pool(name="psB", bufs=2, space="PSUM") as psB,
            tc.tile_pool(name="psS", bufs=2, space="PSUM") as psS,
        ):
            # ---- persistent SBUF tensors ----
            wq_sb = constp.tile([P, DMC, DHH], bf16, tag="wq")
            wk_sb = constp.tile([P, DMC, DHH], bf16, tag="wk")
            wv_sb = constp.tile([P, DMC, DHH], bf16, tag="wv")
            wo_sb = constp.tile([P, 4, D], bf16, tag="wo")
            QT = constp.tile([P, 4, S], bf16, tag="QT")
            KT = constp.tile([P, 4, S], bf16, tag="KT")
            V = constp.tile([P, NT, HPC, DH + 1], bf16, tag="V")
            AT = constp.tile([P, 4, S], bf16, tag="AT")
            vpad = constp.tile([P, NT], f32, tag="vpad")
            mask = constp.tile([P, P], f32, tag="mask")
            bq_sb = constp.tile([P, 4], f32, tag="bq")
            bk_sb = constp.tile([P, 4], f32, tag="bk")
            sel8 = constp.tile([HPC, 4, P], bf16, tag="sel8")

            nc.sync.dma_start(wq_sb[:], wq_d.rearrange("(c p) m -> p c m", p=P))
            # pad-indicator column: the softmax-denominator "ones" column,
            # zeroed for padded key rows
            nc.sync.dma_start(V[:, :, :, DH : DH + 1], vpadc_d[:])

            def load_x(qb):
                # split q into halves so the first projection chains can
                # start before the whole 1MB q tile lands
                qinA = inp.tile([P, DMC // 2, QB], bf16, tag="qinA", name="qinA")
                qinB = inp.tile([P, DMC // 2, QB], bf16, tag="qinB", name="qinB")
                kin = inp.tile([P, DMC, QB], bf16, tag="kin", name="kin")
                vin = inp.tile([P, DMC, QB], bf16, tag="vin", name="vin")
                x0v = x0_d[qb].rearrange("t (c p) s -> t p c s", p=P)
                nc.sync.dma_start(qinA[:], x0v[0, :, 0 : DMC // 2])
                nc.sync.dma_start(qinB[:], x0v[0, :, DMC // 2 : DMC])
                if qb == 0:
                    nc.sync.dma_start(wk_sb[:], wk_d.rearrange("(c p) m -> p c m", p=P))
                nc.sync.dma_start(kin[:], x0v[1])
                if qb == 0:
                    nc.sync.dma_start(wv_sb[:], wv_d.rearrange("(c p) m -> p c m", p=P))
                nc.sync.dma_start(vin[:], x0v[2])
                if qb == 0:
                    nc.sync.dma_start(vpad[:], vpad_d[:])
                    nc.sync.dma_start(mask[:], mask_d[:])
                    nc.sync.dma_start(bq_sb[:], bq_d[:])
                    nc.sync.dma_start(bk_sb[:], bk_d[:])
                    nc.sync.dma_start(sel8[:], sel8_d[:])
                if qb == 1:
                    nc.sync.dma_start(
                        wo_sb[:], wo_d.rearrange("(c p) m -> p c m", p=P)
                    )
                return (qinA, qinB), kin, vin

            def phase_a(qb, xts):
                q0 = qb * QB
                kn = min(QB, max(0, kcap * P - q0))
                (qinA, qinB), kin, vin = xts

                for c in range(4):
                    ps = psAC.tile([P, QB], f32, tag="proj", name="ps")
                    for mc in range(DMC):
                        qsrc = qinA if mc < DMC // 2 else qinB
                        nc.tensor.matmul(
                            ps[:],
                            wq_sb[:, mc, c * P : (c + 1) * P],
                            qsrc[:, mc % (DMC // 2), :],
                            start=(mc == 0),
                            stop=(mc == DMC - 1),
                        )
                    nc.vector.tensor_scalar(
                        QT[:, c, q0 : q0 + QB], ps[:], bq_sb[:, c : c + 1], None, ADD
                    )
                    if kn > 0:
                        ps = psAC.tile([P, QB], f32, tag="proj", name="ps")
                        for mc in range(DMC):
                            nc.tensor.matmul(
                                ps[:, :kn],
                                wk_sb[:, mc, c * P : (c + 1) * P],
                                kin[:, mc, :kn],
                                start=(mc == 0),
                                stop=(mc == DMC - 1),
                            )
                        nc.vector.tensor_scalar(
                            KT[:, c, q0 : q0 + kn], ps[:, :kn], bk_sb[:, c : c + 1], None, ADD
                        )
                    # interleave one V k-tile per c so B(qb)'s first PV work
                    # becomes ready early
                    kt_i = 4 * qb + c
                    if kt_i >= kcap:
                        continue
                    ps = psAC.tile([P, QB], f32, tag="proj", name="ps")
                    for mc in range(DMC):
                        nc.tensor.matmul(
                            ps[:],
                            vin[:, mc, c * P : (c + 1) * P],
                            wv_sb[:, mc, :],
                            start=(mc == 0),
                            stop=(mc == DMC - 1),
                        )
                    # V copy fused with the key-padding zeroing
                    nc.vector.tensor_scalar(
                        V[:, kt_i, :, 0:DH],
                        ps[:].rearrange("p (h d) -> p h d", d=DH),
                        vpad[:, kt_i : kt_i + 1],
                        None,
                        MUL,
                    )

            def phase_b(qb, den):
                q0 = qb * QB
                kt_max = min(4 * qb + 4, kcap)
                # pair up k-tiles: each pair shares one 2-bank PSUM tile and
                # one exp instruction
                pairs = []
                kt = 0
                while kt < kt_max:
                    if kt + 1 < kt_max:
                        pairs.append((kt, kt + 1))
                        kt += 2
                    else:
                        pairs.append((kt, None))
                        kt += 1
                # Software pipeline: the PV matmuls for a pair are emitted
                # TRAIL pair-steps after its exp, so every PV dispatches with
                # multi-microsecond slack on the ACT result instead of
                # stalling the PE stream (a PE gap costs its duration plus a
                # ~3us half-clock p-state recovery).
                TRAIL = min(3, len(pairs))
                pending = []
                pv_tiles = {}

                def finalize_head(h, pv):
                    po = (h % 2) * DH
                    ch = h // 2
                    # denominator row: PSUM -> SBUF bounce (DMA cannot read
                    # PSUM), then DMA into the den row for this head
                    dtmp = smallp.tile([DH + 1, QB], f32, tag="dtmp", name="dtmp")
                    nc.vector.tensor_copy(dtmp[DH : DH + 1, :], pv[DH : DH + 1, :])
                    nc.sync.dma_start(den[h : h + 1, :], dtmp[DH : DH + 1, :])
                    if po == 0:
                        nc.vector.tensor_copy(AT[0:DH, ch, q0 : q0 + QB], pv[0:DH, :])
                    else:
                        tmp = smallp.tile([DH, QB], bf16, tag="tmp", name="tmp")
                        nc.vector.tensor_copy(tmp[:], pv[0:DH, :])
                        nc.sync.dma_start(AT[po : po + DH, ch, q0 : q0 + QB], tmp[:])

                def emit_pv(ent):
                    h, ka, kb, pr = ent
                    if h not in pv_tiles:
                        pv_tiles[h] = psB.tile(
                            [DH + 1, QB], f32, tag="pv", name="pv"
                        )
                    pv = pv_tiles[h]
                    last = False
                    for idx, kt_i in ((0, ka), (1, kb)):
                        if kt_i is None:
                            continue
                        dd = max(0, P * kt_i - q0)
                        nc.tensor.matmul(
                            pv[:, dd:],
                            V[:, kt_i, h, :],
                            pr[:, idx, dd:],
                            start=(kt_i == 0),
                            stop=(kt_i == kt_max - 1),
                        )
                        last = last or (kt_i == kt_max - 1)
                    if last:
                        finalize_head(h, pv)

                for h in range(HPC):
                    po = (h % 2) * DH
                    ch = h // 2
                    for ka, kb in pairs:
                        da = max(0, P * ka - q0)
                        scp = psS.tile([P, 2, QB], f32, tag="sc", name="scp")
                        for idx, kt_i in ((0, ka), (1, kb)):
                            if kt_i is None:
                                continue
                            dd = max(0, P * kt_i - q0)
                            nc.tensor.matmul(
                                scp[:, idx, dd:],
                                KT[po : po + DH, ch, kt_i * P : (kt_i + 1) * P],
                                QT[po : po + DH, ch, q0 + dd : q0 + QB],
                                start=True,
                                stop=True,
                            )
                            if kt_i >= 4 * qb:
                                nc.vector.tensor_tensor(
                                    scp[:, idx, dd : dd + P],
                                    scp[:, idx, dd : dd + P],
                                    mask[:],
                                    ADD,
                                )
                        pr = probsp.tile([P, 2, QB], bf16, tag="probs", name="pr")
                        if kb is not None:
                            nc.scalar.activation(
                                pr[:, :, da:], scp[:, :, da:], EXP, scale=0.125
                            )
                        else:
                            nc.scalar.activation(
                                pr[:, 0, da:], scp[:, 0, da:], EXP, scale=0.125
                            )
                        pending.append((h, ka, kb, pr))
                        if len(pending) > TRAIL:
                            emit_pv(pending.pop(0))
                while pending:
                    emit_pv(pending.pop(0))

            def normalize(qb, den, recf, recd):
                q0 = qb * QB
                nc.vector.reciprocal_approx_fast(recf[:], den[:])
                with nc.allow_low_precision(reason="bf16 recip for bf16 attn"):
                    nc.vector.tensor_copy(recd[:], recf[:])
                for ch in range(4):
                    rep = psAC.tile([P, QB], f32, tag="proj", name="rep")
                    nc.tensor.matmul(
                        rep[:], sel8[:, ch, :], recd[:], start=True, stop=True
                    )
                    nc.vector.tensor_tensor(
                        AT[:, ch, q0 : q0 + QB],
                        AT[:, ch, q0 : q0 + QB],
                        rep[:],
                        MUL,
                    )

            def phase_c(qb):
                for j in range(4):
                    qt_i = 4 * qb + j
                    osb = osbp.tile([P, D], bf16, tag="osb", name="osb")
                    for half in range(2):
                        fin = psAC.tile([P, QB], f32, tag="proj", name="fin")
                        for c in range(4):
                            lhsT = AT[:, c, qt_i * P : (qt_i + 1) * P]
                            nc.tensor.matmul(
                                fin[:],
                                lhsT,
                                wo_sb[:, c, half * 512 : half * 512 + 512],
                                start=(c == 0),
                                stop=(c == 3),
                            )
                        nc.vector.tensor_copy(
                            osb[:, half * 512 : half * 512 + 512], fin[:]
                        )
                    nc.sync.dma_start(out_d[qt_i * P : (qt_i + 1) * P, :], osb[:])

            xts0 = load_x(0)
            phase_a(0, xts0)
            for qb in range(NQB):
                den = smallp.tile([HPC, QB], f32, tag=f"den{qb % 2}", name="den")
                recf = smallp.tile([HPC, QB], f32, tag=f"recf{qb % 2}", name="recf")
                recd = smallp.tile([HPC, QB], bf16, tag=f"recd{qb % 2}", name="recd")
                phase_b(qb, den)
                if qb + 1 < NQB:
                    xts = load_x(qb + 1)
                    phase_a(qb + 1, xts)
                normalize(qb, den, recf, recd)
                phase_c(qb)

    nc.compile()
    return nc


def _get_program(kcap=NT):
    if kcap not in _CACHE:
        _CACHE[kcap] = _build_program(kcap=kcap)
    return _CACHE[kcap]


def _sel8_const():
    bf = ml_dtypes.bfloat16
    sel8 = np.zeros((HPC, 4, P), dtype=bf)
    for ch in range(4):
        sel8[2 * ch, ch, 0:DH] = 1.0
        sel8[2 * ch + 1, ch, DH:P] = 1.0
    return sel8


def _make_in_maps(q_input, k_input, v_input, key_padding_mask, Wq, Wk, Wv, Wo, bq, bk):
    bf = ml_dtypes.bfloat16
    mask128 = np.where(
        np.arange(P)[None, :] < np.arange(P)[:, None], NEG_CAUSAL, 0.0
    ).astype(np.float32)
    sel8 = _sel8_const()
    xs = (q_input, k_input, v_input)
    in_maps = []
    for core in range(8):
        b = core // 2
        hg = core % 2
        sl = slice(hg * DHH, (hg + 1) * DHH)
        # key-padding as V-side zeroing: padded key rows contribute 0 to
        # both the PV numerator and the ones-column denominator
        keep = (~key_padding_mask[b]).astype(np.float32)  # [S]
        vpad = np.ascontiguousarray(keep.reshape(NT, P).T)  # [P, NT]
        vpadc = np.ascontiguousarray(
            np.broadcast_to(vpad[:, :, None, None], (P, NT, HPC, 1))
        ).astype(bf)
        x0 = np.empty((NQB, 3, D, QB), dtype=bf)
        for t in range(3):
            xT = xs[t][b].T  # [D, S] view
            for j in range(NQB):
                x0[j, t] = xT[:, j * QB : (j + 1) * QB]
        in_maps.append(
            {
                "x0": x0,
                "wq": Wq[:, sl].astype(bf),
                "wk": Wk[:, sl].astype(bf),
                "wv": Wv[:, sl].astype(bf),
                "wo": np.ascontiguousarray(Wo[sl, :]).astype(bf),
                "vpad": vpad,
                "vpadc": vpadc,
                "mask": mask128,
                "bq": np.ascontiguousarray(bq[sl].reshape(4, P).T.astype(np.float32)),
                "sel8": sel8,
                "bk": np.ascontiguousarray(bk[sl].reshape(4, P).T.astype(np.float32)),
            }
        )
    return in_maps


def run_spmd(in_maps, kcap=NT, **kwargs):
    from concourse import bass_utils

    nc = _get_program(kcap=kcap)
    return bass_utils.run_bass_kernel_spmd(
        nc, in_maps, core_ids=list(range(8)), **kwargs
    )


def kernel(q_input, k_input, v_input, key_padding_mask,
           Wq, bq, Wk, bk, Wv, bv, Wo, bo, **_unused):
    q_input = np.asarray(q_input, dtype=np.float32)
    k_input = np.asarray(k_input, dtype=np.float32)
    v_input = np.asarray(v_input, dtype=np.float32)
    key_padding_mask = np.asarray(key_padding_mask)
    in_maps = _make_in_maps(
        q_input, k_input, v_input, key_padding_mask,
        np.asarray(Wq, np.float32), np.asarray(Wk, np.float32),
        np.asarray(Wv, np.float32), np.asarray(Wo, np.float32),
        np.asarray(bq, np.float32), np.asarray(bk, np.float32),
    )
    valid = S - key_padding_mask.astype(np.int64).sum(axis=1)
    kcap = int(min(NT, max(1, -(-int(valid.max()) // P))))
    res = run_spmd(in_maps, kcap=kcap).results
    bo = np.asarray(bo, np.float32)
    bv = np.asarray(bv, np.float32)
    # bv support: normalized attention plus bv equals attn output with biased V
    # (rows of softmax sum to 1) -> fold bv through Wo into the output bias.
    extra = bv @ np.asarray(Wo, np.float32) if np.any(bv) else 0.0
    out = np.empty((4, S, D), np.float32)
    for b in range(4):
        out[b] = res[2 * b]["outp"].astype(np.float32)
        out[b] += res[2 * b + 1]["outp"].astype(np.float32)
    out += bo + extra
    return out
